# revision 2
# baseline (speedup 1.0000x reference)
"""Full-input kernel entry: shards across 8 NeuronCores as (batch x head-half),
runs the Bass attention kernel, gathers + reduces partials on host.

Dev version: imports attn_core. The graded submission inlines it.
"""

import numpy as np
from contextlib import ExitStack

import concourse.tile as tile
from concourse import bacc
from concourse.bass_utils import run_bass_kernel_spmd

import attn_core

B, T, C, H = 4, 2048, 1024, 16
N_CORES = 8
HG = 2                      # head groups (tensor-parallel axis)
NW = C // HG                # 512 columns of W_k per group

_cache = {}


def get_compiled(dt_s=attn_core.BF16, dt_v=attn_core.BF16):
    key = (dt_s, dt_v)
    if key not in _cache:
        cfg = attn_core.Cfg(T=T, CIN=C, HL=H // HG, COUT=C, dt_s=dt_s, dt_v=dt_v)
        nc = bacc.Bacc("TRN2", target_bir_lowering=False, debug=False,
                       num_devices=N_CORES)
        io = attn_core.declare_io(nc, cfg)
        with tile.TileContext(nc) as tc:
            with ExitStack() as ctx:
                attn_core.build(ctx, tc, io, cfg)
        nc.compile()
        _cache[key] = (nc, cfg)
    return _cache[key]


def make_in_maps(cfg, x, W_attn, b_attn, W_proj):
    in_maps = []
    for core in range(N_CORES):
        b, hg = core // HG, core % HG
        sl = slice(C + hg * NW, C + (hg + 1) * NW)
        in_maps.append(attn_core.make_inputs(
            cfg, x[b], W_attn[:, sl], b_attn[sl],
            W_proj[hg * NW:(hg + 1) * NW, :]))
    return in_maps


def kernel(x, W_attn, b_attn, W_proj, b_proj):
    x = np.asarray(x, dtype=np.float32)
    W_attn = np.asarray(W_attn, dtype=np.float32)
    b_attn = np.asarray(b_attn, dtype=np.float32)
    W_proj = np.asarray(W_proj, dtype=np.float32)
    b_proj = np.asarray(b_proj, dtype=np.float32)

    nc, cfg = get_compiled()
    in_maps = make_in_maps(cfg, x, W_attn, b_attn, W_proj)
    res = run_bass_kernel_spmd(nc, in_maps, core_ids=list(range(N_CORES)))
    out = np.empty((B, T, C), dtype=np.float32)
    for b in range(B):
        out[b] = res.results[HG * b]["out"] + res.results[HG * b + 1]["out"] \
            + b_proj[None, :]
    return out
